# revision 43
# baseline (speedup 1.0000x reference)
"""Trainium2 Bass kernel for GQA multi-head attention block (nn_MHA_68831145886222).

Computation (reference):
  qkv = x @ w_qkv.T ; split q[32 heads],k[8],v[8] (HD=128)
  q,k = rmsnorm(head_dim) -> rope(interleaved, theta=1e6)
  out = causal GQA attention (4 q heads per kv head)
  y   = (attn out) @ w_out.T

Sharding: tensor-parallel by kv-head group. Core g of 8 owns q heads
4g..4g+3 and kv head g (columns of the qkv projection), plus the matching
512 input rows of w_out. Each core computes a partial y [2048,4096]; the
host sums the 8 partials (bf16 partials, f32 host accumulate).

v2 changes vs v1 (423us):
  - softmax denominator no longer uses a ones-matmul per k-tile (was ~8% of
    all PE cycles): exp tiles are accumulated on the vector engine into an
    f32 per-partition partial sum; one ones-matmul per (head, q-chunk)
    reduces over partitions at the end.
  - stage-1 ramp: s-tiles 0/1 are computed interleaved per d-tile, with
    x/wq DMAs issued in consumption order, so the first matmul starts at
    ~7us instead of ~22us.
  - w_out is DMA'd in 4 e-chunks (aliased over the wq SBUF region) so the
    first out-proj tile doesn't wait for the full 4MB load.
  - output written as bf16 (halves out DMA; host sums in f32), final
    s-tile's DMA split in quarters to shrink the drain tail.
"""

import os
import sys
import types

import numpy as np

H = 32
G = 8
HD = 128
S = 2048
D = 4096
HG = H // G  # q heads per kv head = 4
EPS = 1e-5
THETA = 1e6
N_CORES = 8
ST = S // 128  # 16 s-tiles
DT = D // 128  # 32 d-tiles
QC = 4  # q chunks of 512
EC = 8  # e chunks of 512 in final matmul


def _ensure_ntff_hook():
    """Register the axon NTFF profile hook if the image's antenv lacks it,
    so run_bass_kernel_spmd(trace=True) can return exec_time_ns."""
    try:
        from antenv.axon_hooks import get_axon_ntff_profile_hook  # noqa: F401
        return
    except ImportError:
        pass
    try:
        import antenv
        mod = types.ModuleType("antenv.axon_hooks")
        _h = [None]
        mod.set_axon_ntff_profile_hook = lambda h: _h.__setitem__(0, h)
        mod.get_axon_ntff_profile_hook = lambda: _h[0]
        sys.modules["antenv.axon_hooks"] = mod
        antenv.axon_hooks = mod
        from trn_agent_boot.trn_boot import _ntff_profile_via_ctypes
        so = "/opt/axon/libaxon_pjrt.so"
        if os.path.exists(so):
            mod.set_axon_ntff_profile_hook(_ntff_profile_via_ctypes(so))
    except Exception:
        pass


def _build_nc():
    import concourse.bass as bass  # noqa: F401
    import concourse.tile as tile
    from concourse import bacc, mybir

    bf16 = mybir.dt.bfloat16
    f16 = mybir.dt.float16
    f32 = mybir.dt.float32
    AF = mybir.ActivationFunctionType

    nc = bacc.Bacc("TRN2", target_bir_lowering=False, debug=False,
                   num_devices=N_CORES)

    # ---- DRAM I/O ----
    xt_d = nc.dram_tensor("xt", [ST, 128, DT, 128], bf16, kind="ExternalInput").ap()
    wqkv_d = nc.dram_tensor("wqkvT", [D, 768], bf16, kind="ExternalInput").ap()
    wo_d = nc.dram_tensor("woT", [512, D], bf16, kind="ExternalInput").ap()
    ccd_d = nc.dram_tensor("ccd", [S, 128], f16, kind="ExternalInput").ap()
    ssd_d = nc.dram_tensor("ssd", [S, 128], f16, kind="ExternalInput").ap()
    mask_d = nc.dram_tensor("dmask", [128, 128], bf16, kind="ExternalInput").ap()
    ident_d = nc.dram_tensor("ident", [128, 128], bf16, kind="ExternalInput").ap()
    out_d = nc.dram_tensor("out", [S, D], bf16, kind="ExternalOutput").ap()

    from contextlib import ExitStack
    with tile.TileContext(nc) as tc, ExitStack() as ctx:
        const = ctx.enter_context(tc.tile_pool(name="const", bufs=1))
        persist = ctx.enter_context(tc.tile_pool(name="persist", bufs=1))
        xpool = ctx.enter_context(tc.tile_pool(name="xpool", bufs=4))
        scratch = ctx.enter_context(tc.tile_pool(name="scratch", bufs=2))
        small = ctx.enter_context(tc.tile_pool(name="small", bufs=2))
        epool = ctx.enter_context(tc.tile_pool(name="epool", bufs=9))
        apool = ctx.enter_context(tc.tile_pool(name="apool", bufs=4))
        opool = ctx.enter_context(tc.tile_pool(name="opool", bufs=2))
        psum = ctx.enter_context(tc.tile_pool(name="psum", bufs=4, space="PSUM"))

        # ---- DMA issue order = consumption order: s-tiles 0/1 compute
        # interleaved per d-tile group so the PE starts as soon as the first
        # x chunk + wq d-slice land.
        wq_sb = persist.tile([128, DT, 768], bf16, tag="bigw")
        wq_r = wqkv_d.rearrange("(t p) e -> p t e", p=128)

        xs0 = xpool.tile([128, DT, 128], bf16, name="xs")
        xs1 = xpool.tile([128, DT, 128], bf16, name="xs")
        xs2 = xpool.tile([128, DT, 128], bf16, name="xs")
        # first matmul needs only xs0 d-tile 0 + wq[0, q-cols]; everything
        # else streams just-in-time in d-tile-group order for the 3-way
        # interleaved ramp (PE ~3.84us/group vs DMA ~3.1us/group).
        nc.sync.dma_start(out=xs0[:, 0:1, :], in_=xt_d[0, :, 0:1, :])
        nc.sync.dma_start(out=wq_sb[:, 0:1, 0:512], in_=wq_r[:, 0:1, 0:512])
        nc.sync.dma_start(out=xs1[:, 0:1, :], in_=xt_d[1, :, 0:1, :])
        nc.sync.dma_start(out=xs2[:, 0:1, :], in_=xt_d[2, :, 0:1, :])
        nc.sync.dma_start(out=wq_sb[:, 0:1, 512:768], in_=wq_r[:, 0:1, 512:768])
        nc.sync.dma_start(out=xs0[:, 1:4, :], in_=xt_d[0, :, 1:4, :])
        nc.sync.dma_start(out=xs1[:, 1:4, :], in_=xt_d[1, :, 1:4, :])
        nc.sync.dma_start(out=xs2[:, 1:4, :], in_=xt_d[2, :, 1:4, :])
        nc.sync.dma_start(out=wq_sb[:, 1:2, :], in_=wq_r[:, 1:2, :])
        nc.sync.dma_start(out=wq_sb[:, 2:4, :], in_=wq_r[:, 2:4, :])

        ccd_sb = const.tile([128, ST, 128], f16)
        ssd_sb = const.tile([128, ST, 128], f16)
        mask_sb = const.tile([128, 128], bf16)
        ident_sb = const.tile([128, 128], bf16)
        for g in range(4, DT, 4):
            nc.sync.dma_start(out=xs0[:, g:g + 4, :], in_=xt_d[0, :, g:g + 4, :])
            nc.sync.dma_start(out=xs1[:, g:g + 4, :], in_=xt_d[1, :, g:g + 4, :])
            nc.sync.dma_start(out=xs2[:, g:g + 4, :], in_=xt_d[2, :, g:g + 4, :])
            nc.sync.dma_start(out=wq_sb[:, g:g + 4, :], in_=wq_r[:, g:g + 4, :])
        nc.sync.dma_start(out=ccd_sb,
                          in_=ccd_d.rearrange("(t p) h -> p t h", p=128))
        nc.sync.dma_start(out=ssd_sb,
                          in_=ssd_d.rearrange("(t p) h -> p t h", p=128))
        nc.sync.dma_start(out=mask_sb, in_=mask_d)
        nc.sync.dma_start(out=ident_sb, in_=ident_d)

        onesm_sb = const.tile([128, 128], bf16)
        nc.vector.memset(onesm_sb, 1.0)
        bias_q = const.tile([128, 1], f32)
        nc.vector.memset(bias_q, float(HD * EPS))
        bias_k = const.tile([128, 1], f32)
        nc.vector.memset(bias_k, float(EPS))

        qT_sb = persist.tile([128, QC, HG, 512], bf16)  # [hd, qchunk, head, s%512]
        kT_sb = persist.tile([128, S], bf16)       # [hd, s]
        v_sb = persist.tile([128, ST, 128], bf16)  # [s_local, s_tile, hd]
        oT_sb = persist.tile([128, HG, S], bf16)   # attn outT [hd, head, s]

        ALU = mybir.AluOpType

        def pp_k(st, kv_ps):
            # k/v part of the postproc: v copy, k rmsnorm + rope -> kfin.
            # split from the q part so the last s-tile can release its kv
            # psum slot before attention q-chunk 0 is emitted.
            nc.vector.tensor_copy(out=v_sb[:, st, :], in_=kv_ps[:, 128:256])

            k1 = kv_ps[:, 0:128].rearrange("p (r two) -> p r two", two=2)
            rot_k = small.tile([128, 64, 2], f32)
            nc.vector.tensor_copy(out=rot_k, in_=k1[:, :, ::-1])
            kcc = small.tile([128, 128], f32)
            nc.vector.tensor_mul(kcc, kv_ps[:, 0:128], ccd_sb[:, st, :])
            kss = small.tile([128, 128], f32)
            nc.vector.tensor_mul(kss, rot_k.rearrange("p r two -> p (r two)"),
                                 ssd_sb[:, st, :])
            krope = small.tile([128, 128], f32)
            nc.vector.tensor_add(krope, kcc, kss)
            # rope is an orthogonal rotation -> per-head norm is unchanged,
            # so the rmsnorm stats can come from the SBUF roped values:
            # the kv psum slot is released as soon as the rope reads finish
            # instead of waiting on the ACT square queue.
            ssq_k = small.tile([128, 1], f32)
            sqk = small.tile([128, 128], f32)
            nc.scalar.activation(out=sqk, in_=krope, func=AF.Square,
                                 accum_out=ssq_k)
            rstd_k = small.tile([128, 1], f32)
            nc.scalar.activation(out=rstd_k, in_=ssq_k,
                                 func=AF.Sqrt, bias=bias_k, scale=1.0 / HD)
            nc.vector.reciprocal(out=rstd_k, in_=rstd_k)
            kfin = small.tile([128, 128], bf16)
            nc.vector.tensor_scalar_mul(kfin, krope, rstd_k)
            return kfin

        def pp_q(st, q_ps):
            # q part: rope first, then per-head rmsnorm stats (with folded
            # score scale) from the SBUF roped values (rotation preserves
            # the norm), keeping the squares on DVE instead of ACT.
            q4 = q_ps.rearrange("p (h r two) -> p h r two", h=HG, two=2)
            rot_q = scratch.tile([128, HG, 64, 2], f32)
            nc.vector.tensor_copy(out=rot_q, in_=q4[:, :, :, ::-1])
            cc_b = ccd_sb[:, st, :].unsqueeze(1).broadcast_to((128, HG, 128))
            ss_b = ssd_sb[:, st, :].unsqueeze(1).broadcast_to((128, HG, 128))
            qcc = scratch.tile([128, HG, 128], f32)
            nc.vector.tensor_mul(qcc, q_ps.rearrange("p (h e) -> p h e", h=HG), cc_b)
            qss = scratch.tile([128, HG, 128], f32)
            nc.vector.tensor_mul(qss, rot_q.rearrange("p h r two -> p h (r two)"), ss_b)
            qrope = scratch.tile([128, HG, 128], f32)
            nc.vector.tensor_add(qrope, qcc, qss)

            ssq = small.tile([128, 4], f32)
            sqs = scratch.tile([128, 512], f32)
            for hh in range(HG):
                nc.scalar.activation(out=sqs[:, hh * 128:(hh + 1) * 128],
                                     in_=qrope[:, hh, :],
                                     func=AF.Square,
                                     accum_out=ssq[:, hh:hh + 1])
            rstd = small.tile([128, 4], f32)
            nc.scalar.activation(out=rstd, in_=ssq,
                                 func=AF.Sqrt, bias=bias_q, scale=1.0)
            nc.vector.reciprocal(out=rstd, in_=rstd)
            qfin = scratch.tile([128, HG, 128], bf16)
            for hh in range(HG):
                nc.vector.tensor_scalar_mul(qfin[:, hh, :], qrope[:, hh, :],
                                            rstd[:, hh:hh + 1])
            return qfin

        def pp_transpose(st, qfin, kfin):
            # transpose q heads and k into [hd, s] layout (PE); emitted one
            # s-tile behind the matmul blocks so the PE never waits on qfin.
            # qT layout groups the 4 heads of each q-chunk adjacently so a
            # score matmul pair for two heads can share one psum tile.
            qc_i, so = st // 4, (st % 4) * 128
            for hh in range(HG):
                tq_ps = psum.tile([128, 128], bf16, tag="pd", bufs=2)
                nc.tensor.transpose(tq_ps, qfin[:, hh, :], ident_sb)
                nc.scalar.copy(out=qT_sb[:, qc_i, hh, so:so + 128], in_=tq_ps)
            tk_ps = psum.tile([128, 128], bf16, tag="pd", bufs=2)
            nc.tensor.transpose(tk_ps, kfin, ident_sb)
            nc.scalar.copy(out=kT_sb[:, st * 128:(st + 1) * 128], in_=tk_ps)

        # ================= stage 1: qkv projection + postproc ==============
        # s-tiles 0/1/2 interleaved per d-tile: PE-paced ramp (the DMA
        # stream stays ahead after the first d-tile group)
        q_pre = [psum.tile([128, 512], f32, tag="pa", bufs=3, name=f"q_ps_{i}")
                 for i in range(3)]
        kv_pre = [psum.tile([128, 512], f32, tag="pc", bufs=3, name=f"kv_ps_{i}")
                  for i in range(3)]
        xs_pre = [xs0, xs1, xs2]
        for dt_i in range(DT):
            st_flag = (dt_i == 0)
            sp_flag = (dt_i == DT - 1)
            for i in range(3):
                nc.tensor.matmul(q_pre[i], xs_pre[i][:, dt_i, :],
                                 wq_sb[:, dt_i, 0:512],
                                 start=st_flag, stop=sp_flag)
                nc.tensor.matmul(kv_pre[i][:, 0:256], xs_pre[i][:, dt_i, :],
                                 wq_sb[:, dt_i, 512:768],
                                 start=st_flag, stop=sp_flag)

        qk_fin = {}
        for i in range(3):
            qk_fin[i] = (pp_q(i, q_pre[i]), pp_k(i, kv_pre[i]))

        # s-tiles 3..15 sequential; postproc compute (ACT/DVE) emitted right
        # after each tile's matmuls, PE transposes one tile behind. For the
        # last tile the q part is deferred past attention q-chunk 0 so the
        # seam isn't serialized on it.
        for st in range(3, ST):
            xs = xpool.tile([128, DT, 128], bf16, name="xs")
            nc.sync.dma_start(out=xs, in_=xt_d[st])

            q_ps = psum.tile([128, 512], f32, tag="pa", bufs=3, name=f"q_ps_{st}")
            kv_ps = psum.tile([128, 512], f32, tag="pc", bufs=3, name=f"kv_ps_{st}")
            for dt_i in range(DT):
                nc.tensor.matmul(q_ps, xs[:, dt_i, :], wq_sb[:, dt_i, 0:512],
                                 start=(dt_i == 0), stop=(dt_i == DT - 1))
                nc.tensor.matmul(kv_ps[:, 0:256], xs[:, dt_i, :],
                                 wq_sb[:, dt_i, 512:768],
                                 start=(dt_i == 0), stop=(dt_i == DT - 1))
            if st == 3:
                pp_transpose(0, *qk_fin[0])
                pp_transpose(1, *qk_fin[1])
                pp_transpose(2, *qk_fin[2])
            else:
                pp_transpose(st - 1, *qk_fin[st - 1])
                del qk_fin[st - 1]
            if st < ST - 1:
                qk_fin[st] = (pp_q(st, q_ps), pp_k(st, kv_ps))
            else:
                kfin_last = pp_k(st, kv_ps)
                q_ps_last = q_ps

        # ================= stage 3 weights (aliased over wq, 4 chunks) =====
        wo_sb = persist.tile([128, HG, D], bf16, tag="bigw")
        wo_r = wo_d.rearrange("(h p) e -> p h e", p=128)
        for ec2 in range(4):
            nc.sync.dma_start(out=wo_sb[:, :, ec2 * 1024:(ec2 + 1) * 1024],
                              in_=wo_r[:, :, ec2 * 1024:(ec2 + 1) * 1024])

        # ---- out-projection "units": generator yields after each matmul so
        # single matmuls can be interleaved into the attention loop as PE
        # filler while the scalar engine works through the exp backlog.
        PSUM_BUFS = {"pd": 2, "pa": 3, "pc": 3}

        def wout_units(st, tags=("pd",)):
            out_sb = opool.tile([128, D], bf16, name="out_sb")
            for ec in range(EC):
                tag = tags[ec % len(tags)]
                o_ps = psum.tile([128, 512], f32, tag=tag,
                                 bufs=PSUM_BUFS[tag], name="o_ps")
                for h in range(HG):
                    nc.tensor.matmul(o_ps,
                                     oT_sb[:, h, st * 128:(st + 1) * 128],
                                     wo_sb[:, h, ec * 512:(ec + 1) * 512],
                                     start=(h == 0), stop=(h == HG - 1))
                    if h < HG - 1:
                        yield
                nc.vector.tensor_copy(
                    out=out_sb[:, ec * 512:(ec + 1) * 512], in_=o_ps)
                if st == ST - 1:
                    if ec % 2 == 1:
                        nc.sync.dma_start(
                            out=out_d[st * 128:(st + 1) * 128,
                                      (ec - 1) * 512:(ec + 1) * 512],
                            in_=out_sb[:, (ec - 1) * 512:(ec + 1) * 512])
                elif ec == 3 or ec == 7:
                    half = ec // 4
                    nc.sync.dma_start(
                        out=out_d[st * 128:(st + 1) * 128,
                                  half * 2048:(half + 1) * 2048],
                        in_=out_sb[:, half * 2048:(half + 1) * 2048])
                yield

        from collections import deque
        pending = deque()

        def emit_units(n):
            k = 0
            while k < n and pending:
                try:
                    next(pending[0])
                    k += 1
                except StopIteration:
                    pending.popleft()

        # deferred per-head softmax finish: partition-reduce the accumulated
        # exp sums (ones-matmul), reciprocal, scale pv into oT.
        fin_q = deque()

        def mk_finish(h, qc, pv, acc):
            def f():
                # acc is already bf16: ones-matmul reduces it over partitions
                # directly; reciprocal and the pv scale read PSUM -> DVE.
                den_ps = psum.tile([128, 512], f32, tag="pd", bufs=2, name="den")
                nc.tensor.matmul(den_ps, onesm_sb, acc, start=True, stop=True)
                rden = scratch.tile([128, 512], f32, tag="rden")
                nc.vector.reciprocal_approx_fast(out=rden, in_=den_ps)
                nc.vector.tensor_mul(oT_sb[:, h, qc * 512:(qc + 1) * 512],
                                     pv, rden)
            return f

        # ================= stage 2+3 fused ================================
        for qc in range(QC):
            for hp in range(HG // 2):
                hh0 = 2 * hp
                pv0 = psum.tile([128, 512], f32, tag="pa", bufs=3, name=f"pv0_{qc}_{hp}")
                pv1 = psum.tile([128, 512], f32, tag="pa", bufs=3, name=f"pv1_{qc}_{hp}")
                pvs = [pv0, pv1]
                # bf16 accumulators: positive growing sums round benignly
                # (measured den err ~1e-4) and packed bf16 gets the 2x/4x
                # DVE modes, halving the per-iteration accumulate cost.
                accs = [apool.tile([128, 512], bf16, name="acc")
                        for hi in range(2)]
                n_kt = 4 * qc + 4
                for kt in range(n_kt):
                    j = kt - 4 * qc
                    off = 0 if j < 0 else 128 * j
                    exs = []
                    for hi in range(2):
                        sc_ps = psum.tile([128, 512], f32, tag="pc", bufs=3,
                                          name=f"sc_{qc}_{hp}_{kt}_{hi}")
                        nc.tensor.matmul(
                            sc_ps[:, off:512],
                            kT_sb[:, kt * 128:(kt + 1) * 128],
                            qT_sb[:, qc, hh0 + hi, off:512],
                            start=True, stop=True)
                        ex = epool.tile([128, 512], bf16, name=f"ex_{hi}")
                        nc.scalar.activation(out=ex[:, off:512],
                                             in_=sc_ps[:, off:512], func=AF.Exp)
                        if j >= 0:
                            # diag mask on the otherwise-idle GPSIMD (all
                            # SBUF operands) to keep it off the DVE chain
                            nc.gpsimd.tensor_mul(ex[:, off:off + 128],
                                                 ex[:, off:off + 128],
                                                 mask_sb)
                        if kt == 0:
                            nc.vector.tensor_copy(out=accs[hi], in_=ex)
                        else:
                            nc.vector.tensor_add(accs[hi][:, off:512],
                                                 accs[hi][:, off:512],
                                                 ex[:, off:512])
                        exs.append(ex)
                    for hi in range(2):
                        nc.tensor.matmul(pvs[hi][:, off:512], v_sb[:, kt, :],
                                         exs[hi][:, off:512],
                                         start=(kt == 0), stop=(kt == n_kt - 1))
                    # pop pending per-head finishes mid-block (kt 2/3, clear
                    # of the kt0/1 accumulator-init ops), then fill the PE
                    # with out-proj matmuls while ACT works through the exps.
                    if 2 <= kt <= 3 and fin_q:
                        fin_q.popleft()()
                    if hp == 0 and kt == 4 and qc >= 1:
                        for st in range(4 * (qc - 1), 4 * qc):
                            pending.append(wout_units(st))
                    emit_units(2 + max(0, j))
                if qc == 0 and hp == 0:
                    # deferred last stage-1 q-postproc + transpose: nothing
                    # before q-chunk 3 reads s-tile 15.
                    qfin_last = pp_q(ST - 1, q_ps_last)
                    pp_transpose(ST - 1, qfin_last, kfin_last)
                for hi in range(2):
                    fin_q.append(mk_finish(hh0 + hi, qc, pvs[hi], accs[hi]))

        while fin_q:
            fin_q.popleft()()
        # drain: attention psum tags are free now, cycle o_ps across all
        # three rings so chunk allocation never waits on a copy.
        for st in range(4 * (QC - 1), 4 * QC):
            pending.append(wout_units(st, tags=("pd", "pa", "pc")))
        emit_units(1 << 30)


    nc.compile()
    return nc


def _host_prep(x, w_qkv, w_out, q_ln_w, k_ln_w):
    """Build per-core input maps (host-side shard + transform)."""
    import ml_dtypes
    bf16 = ml_dtypes.bfloat16

    x2 = np.asarray(x, np.float32).reshape(S, D)
    # x tiles [st, d_local, d_tile, s_local] so each s-tile DMA is contiguous
    xt = np.ascontiguousarray(
        x2.reshape(ST, 128, DT, 128).transpose(0, 3, 2, 1)).astype(bf16)

    # rope tables (duplicated cos / sign-baked sin, interleaved layout)
    freqs = 1.0 / (THETA ** (np.arange(0, HD, 2, dtype=np.float64) / HD))
    ang = np.arange(S, dtype=np.float64)[:, None] * freqs[None, :]
    cos = np.cos(ang).astype(np.float32)
    sin = np.sin(ang).astype(np.float32)
    ccd = np.repeat(cos, 2, axis=1).astype(np.float16)    # [S, 128]
    ssd = np.stack([-sin, sin], axis=-1).reshape(S, HD).astype(np.float16)

    kq = np.arange(128)
    dmask = (kq[:, None] <= kq[None, :]).astype(bf16)     # [k, q]
    ident = np.eye(128, dtype=bf16)

    wq = np.asarray(w_qkv, np.float32)
    wo = np.asarray(w_out, np.float32)
    qw = np.asarray(q_ln_w, np.float32)
    kw = np.asarray(k_ln_w, np.float32)

    in_maps = []
    for g in range(N_CORES):
        wq_g = wq[512 * g:512 * (g + 1), :].reshape(HG, HD, D) * qw[None, :, None]
        wk_g = wq[D + 128 * g:D + 128 * (g + 1), :] * kw[:, None]
        wv_g = wq[D + G * HD + 128 * g:D + G * HD + 128 * (g + 1), :]
        wqkv_g = np.concatenate([wq_g.reshape(512, D), wk_g, wv_g], axis=0)
        wqkvT_g = np.ascontiguousarray(wqkv_g.T).astype(bf16)     # [D, 768]
        woT_g = np.ascontiguousarray(wo[:, 512 * g:512 * (g + 1)].T).astype(bf16)
        in_maps.append({
            "xt": xt,
            "wqkvT": wqkvT_g,
            "woT": woT_g,
            "ccd": ccd,
            "ssd": ssd,
            "dmask": dmask,
            "ident": ident,
        })
    return in_maps


_CACHE = {}


def _get_compiled():
    if "nc" not in _CACHE:
        _ensure_ntff_hook()
        _CACHE["nc"] = _build_nc()
    return _CACHE["nc"]


def run_sharded(x, w_qkv, w_out, q_ln_w, k_ln_w, trace=False):
    from concourse.bass_utils import run_bass_kernel_spmd
    nc = _get_compiled()
    in_maps = _host_prep(x, w_qkv, w_out, q_ln_w, k_ln_w)
    res = run_bass_kernel_spmd(nc, in_maps, core_ids=list(range(N_CORES)),
                               trace=trace)
    acc = np.zeros((S, D), np.float32)
    for i in range(N_CORES):
        acc += np.asarray(res.results[i]["out"], np.float32)
    return acc.reshape(1, S, D), res


def kernel(x, w_qkv, w_out, q_ln_w, k_ln_w):
    out, _ = run_sharded(x, w_qkv, w_out, q_ln_w, k_ln_w, trace=False)
    return out


# revision 45
# speedup vs baseline: 1.0051x; 1.0051x over previous
"""Trainium2 Bass kernel for GQA multi-head attention block (nn_MHA_68831145886222).

Computation (reference):
  qkv = x @ w_qkv.T ; split q[32 heads],k[8],v[8] (HD=128)
  q,k = rmsnorm(head_dim) -> rope(interleaved, theta=1e6)
  out = causal GQA attention (4 q heads per kv head)
  y   = (attn out) @ w_out.T

Sharding: tensor-parallel by kv-head group. Core g of 8 owns q heads
4g..4g+3 and kv head g (columns of the qkv projection), plus the matching
512 input rows of w_out. Each core computes a partial y [2048,4096]; the
host sums the 8 partials (bf16 partials, f32 host accumulate).

v2 changes vs v1 (423us):
  - softmax denominator no longer uses a ones-matmul per k-tile (was ~8% of
    all PE cycles): exp tiles are accumulated on the vector engine into an
    f32 per-partition partial sum; one ones-matmul per (head, q-chunk)
    reduces over partitions at the end.
  - stage-1 ramp: s-tiles 0/1 are computed interleaved per d-tile, with
    x/wq DMAs issued in consumption order, so the first matmul starts at
    ~7us instead of ~22us.
  - w_out is DMA'd in 4 e-chunks (aliased over the wq SBUF region) so the
    first out-proj tile doesn't wait for the full 4MB load.
  - output written as bf16 (halves out DMA; host sums in f32), final
    s-tile's DMA split in quarters to shrink the drain tail.
"""

import os
import sys
import types

import numpy as np

H = 32
G = 8
HD = 128
S = 2048
D = 4096
HG = H // G  # q heads per kv head = 4
EPS = 1e-5
THETA = 1e6
N_CORES = 8
ST = S // 128  # 16 s-tiles
DT = D // 128  # 32 d-tiles
QC = 4  # q chunks of 512
EC = 8  # e chunks of 512 in final matmul


def _ensure_ntff_hook():
    """Register the axon NTFF profile hook if the image's antenv lacks it,
    so run_bass_kernel_spmd(trace=True) can return exec_time_ns."""
    try:
        from antenv.axon_hooks import get_axon_ntff_profile_hook  # noqa: F401
        return
    except ImportError:
        pass
    try:
        import antenv
        mod = types.ModuleType("antenv.axon_hooks")
        _h = [None]
        mod.set_axon_ntff_profile_hook = lambda h: _h.__setitem__(0, h)
        mod.get_axon_ntff_profile_hook = lambda: _h[0]
        sys.modules["antenv.axon_hooks"] = mod
        antenv.axon_hooks = mod
        from trn_agent_boot.trn_boot import _ntff_profile_via_ctypes
        so = "/opt/axon/libaxon_pjrt.so"
        if os.path.exists(so):
            mod.set_axon_ntff_profile_hook(_ntff_profile_via_ctypes(so))
    except Exception:
        pass


def _build_nc():
    import concourse.bass as bass  # noqa: F401
    import concourse.tile as tile
    from concourse import bacc, mybir

    bf16 = mybir.dt.bfloat16
    f16 = mybir.dt.float16
    f32 = mybir.dt.float32
    AF = mybir.ActivationFunctionType

    nc = bacc.Bacc("TRN2", target_bir_lowering=False, debug=False,
                   num_devices=N_CORES)

    # ---- DRAM I/O ----
    xt_d = nc.dram_tensor("xt", [ST, 128, DT, 128], bf16, kind="ExternalInput").ap()
    wqkv_d = nc.dram_tensor("wqkvT", [D, 768], bf16, kind="ExternalInput").ap()
    wo_d = nc.dram_tensor("woT", [512, D], bf16, kind="ExternalInput").ap()
    ccd_d = nc.dram_tensor("ccd", [S, 128], f16, kind="ExternalInput").ap()
    ssd_d = nc.dram_tensor("ssd", [S, 128], f16, kind="ExternalInput").ap()
    mask_d = nc.dram_tensor("dmask", [128, 128], bf16, kind="ExternalInput").ap()
    ident_d = nc.dram_tensor("ident", [128, 128], bf16, kind="ExternalInput").ap()
    out_d = nc.dram_tensor("out", [S, D], bf16, kind="ExternalOutput").ap()

    from contextlib import ExitStack
    with tile.TileContext(nc) as tc, ExitStack() as ctx:
        const = ctx.enter_context(tc.tile_pool(name="const", bufs=1))
        persist = ctx.enter_context(tc.tile_pool(name="persist", bufs=1))
        xpool = ctx.enter_context(tc.tile_pool(name="xpool", bufs=4))
        scratch = ctx.enter_context(tc.tile_pool(name="scratch", bufs=2))
        small = ctx.enter_context(tc.tile_pool(name="small", bufs=2))
        epool = ctx.enter_context(tc.tile_pool(name="epool", bufs=9))
        apool = ctx.enter_context(tc.tile_pool(name="apool", bufs=4))
        opool = ctx.enter_context(tc.tile_pool(name="opool", bufs=2))
        psum = ctx.enter_context(tc.tile_pool(name="psum", bufs=4, space="PSUM"))

        # ---- DMA issue order = consumption order: s-tiles 0/1 compute
        # interleaved per d-tile group so the PE starts as soon as the first
        # x chunk + wq d-slice land.
        wq_sb = persist.tile([128, DT, 768], bf16, tag="bigw")
        wq_r = wqkv_d.rearrange("(t p) e -> p t e", p=128)

        xs0 = xpool.tile([128, DT, 128], bf16, name="xs")
        xs1 = xpool.tile([128, DT, 128], bf16, name="xs")
        xs2 = xpool.tile([128, DT, 128], bf16, name="xs")
        # DMA issue order = consumption order for the 2-way interleaved ramp
        nc.sync.dma_start(out=xs0[:, 0:4, :], in_=xt_d[0, :, 0:4, :])
        nc.sync.dma_start(out=wq_sb[:, 0:1, :], in_=wq_r[:, 0:1, :])
        nc.sync.dma_start(out=xs1[:, 0:4, :], in_=xt_d[1, :, 0:4, :])
        nc.sync.dma_start(out=wq_sb[:, 1:2, :], in_=wq_r[:, 1:2, :])
        nc.sync.dma_start(out=wq_sb[:, 2:4, :], in_=wq_r[:, 2:4, :])

        ccd_sb = const.tile([128, ST, 128], f16)
        ssd_sb = const.tile([128, ST, 128], f16)
        mask_sb = const.tile([128, 128], bf16)
        ident_sb = const.tile([128, 128], bf16)
        for g in range(4, DT, 4):
            nc.sync.dma_start(out=xs0[:, g:g + 4, :], in_=xt_d[0, :, g:g + 4, :])
            nc.sync.dma_start(out=xs1[:, g:g + 4, :], in_=xt_d[1, :, g:g + 4, :])
            nc.sync.dma_start(out=wq_sb[:, g:g + 4, :], in_=wq_r[:, g:g + 4, :])
        nc.sync.dma_start(out=xs2[:, 0:16, :], in_=xt_d[2, :, 0:16, :])
        nc.sync.dma_start(out=xs2[:, 16:32, :], in_=xt_d[2, :, 16:32, :])
        nc.sync.dma_start(out=ccd_sb,
                          in_=ccd_d.rearrange("(t p) h -> p t h", p=128))
        nc.sync.dma_start(out=ssd_sb,
                          in_=ssd_d.rearrange("(t p) h -> p t h", p=128))
        nc.sync.dma_start(out=mask_sb, in_=mask_d)
        nc.sync.dma_start(out=ident_sb, in_=ident_d)

        onesm_sb = const.tile([128, 128], bf16)
        nc.vector.memset(onesm_sb, 1.0)
        bias_q = const.tile([128, 1], f32)
        nc.vector.memset(bias_q, float(HD * EPS))
        bias_k = const.tile([128, 1], f32)
        nc.vector.memset(bias_k, float(EPS))

        qT_sb = persist.tile([128, QC, HG, 512], bf16)  # [hd, qchunk, head, s%512]
        kT_sb = persist.tile([128, S], bf16)       # [hd, s]
        v_sb = persist.tile([128, ST, 128], bf16)  # [s_local, s_tile, hd]
        oT_sb = persist.tile([128, HG, S], bf16)   # attn outT [hd, head, s]

        ALU = mybir.AluOpType

        def pp_k(st, kv_ps):
            # k/v part of the postproc: v copy, k rmsnorm + rope -> kfin.
            # split from the q part so the last s-tile can release its kv
            # psum slot before attention q-chunk 0 is emitted.
            nc.vector.tensor_copy(out=v_sb[:, st, :], in_=kv_ps[:, 128:256])

            k1 = kv_ps[:, 0:128].rearrange("p (r two) -> p r two", two=2)
            rot_k = small.tile([128, 64, 2], f32)
            nc.vector.tensor_copy(out=rot_k, in_=k1[:, :, ::-1])
            kcc = small.tile([128, 128], f32)
            nc.vector.tensor_mul(kcc, kv_ps[:, 0:128], ccd_sb[:, st, :])
            kss = small.tile([128, 128], f32)
            nc.vector.tensor_mul(kss, rot_k.rearrange("p r two -> p (r two)"),
                                 ssd_sb[:, st, :])
            krope = small.tile([128, 128], f32)
            nc.vector.tensor_add(krope, kcc, kss)
            # rope is an orthogonal rotation -> per-head norm is unchanged,
            # so the rmsnorm stats can come from the SBUF roped values:
            # the kv psum slot is released as soon as the rope reads finish
            # instead of waiting on the ACT square queue.
            ssq_k = small.tile([128, 1], f32)
            sqk = small.tile([128, 128], f32)
            nc.scalar.activation(out=sqk, in_=krope, func=AF.Square,
                                 accum_out=ssq_k)
            rstd_k = small.tile([128, 1], f32)
            nc.scalar.activation(out=rstd_k, in_=ssq_k,
                                 func=AF.Sqrt, bias=bias_k, scale=1.0 / HD)
            nc.vector.reciprocal(out=rstd_k, in_=rstd_k)
            kfin = small.tile([128, 128], bf16)
            nc.vector.tensor_scalar_mul(kfin, krope, rstd_k)
            return kfin

        def pp_q(st, q_ps):
            # q part: rope first, then per-head rmsnorm stats (with folded
            # score scale) from the SBUF roped values (rotation preserves
            # the norm), keeping the squares on DVE instead of ACT.
            q4 = q_ps.rearrange("p (h r two) -> p h r two", h=HG, two=2)
            rot_q = scratch.tile([128, HG, 64, 2], f32)
            nc.vector.tensor_copy(out=rot_q, in_=q4[:, :, :, ::-1])
            cc_b = ccd_sb[:, st, :].unsqueeze(1).broadcast_to((128, HG, 128))
            ss_b = ssd_sb[:, st, :].unsqueeze(1).broadcast_to((128, HG, 128))
            qcc = scratch.tile([128, HG, 128], f32)
            nc.vector.tensor_mul(qcc, q_ps.rearrange("p (h e) -> p h e", h=HG), cc_b)
            qss = scratch.tile([128, HG, 128], f32)
            nc.vector.tensor_mul(qss, rot_q.rearrange("p h r two -> p h (r two)"), ss_b)
            qrope = scratch.tile([128, HG, 128], f32)
            nc.vector.tensor_add(qrope, qcc, qss)

            ssq = small.tile([128, 4], f32)
            sqs = scratch.tile([128, 512], f32)
            for hh in range(HG):
                nc.scalar.activation(out=sqs[:, hh * 128:(hh + 1) * 128],
                                     in_=qrope[:, hh, :],
                                     func=AF.Square,
                                     accum_out=ssq[:, hh:hh + 1])
            rstd = small.tile([128, 4], f32)
            nc.scalar.activation(out=rstd, in_=ssq,
                                 func=AF.Sqrt, bias=bias_q, scale=1.0)
            nc.vector.reciprocal(out=rstd, in_=rstd)
            qfin = scratch.tile([128, HG, 128], bf16)
            for hh in range(HG):
                nc.vector.tensor_scalar_mul(qfin[:, hh, :], qrope[:, hh, :],
                                            rstd[:, hh:hh + 1])
            return qfin

        def pp_transpose(st, qfin, kfin):
            # transpose q heads and k into [hd, s] layout (PE); emitted one
            # s-tile behind the matmul blocks so the PE never waits on qfin.
            # qT layout groups the 4 heads of each q-chunk adjacently so a
            # score matmul pair for two heads can share one psum tile.
            qc_i, so = st // 4, (st % 4) * 128
            for hh in range(HG):
                tq_ps = psum.tile([128, 128], bf16, tag="pd", bufs=2)
                nc.tensor.transpose(tq_ps, qfin[:, hh, :], ident_sb)
                nc.scalar.copy(out=qT_sb[:, qc_i, hh, so:so + 128], in_=tq_ps)
            tk_ps = psum.tile([128, 128], bf16, tag="pd", bufs=2)
            nc.tensor.transpose(tk_ps, kfin, ident_sb)
            nc.scalar.copy(out=kT_sb[:, st * 128:(st + 1) * 128], in_=tk_ps)

        # ================= stage 1: qkv projection + postproc ==============
        # s-tiles 0/1 interleaved per d-tile (DMA-paced ramp)
        q_ps0 = psum.tile([128, 512], f32, tag="pa", bufs=3, name="q_ps_0")
        q_ps1 = psum.tile([128, 512], f32, tag="pa", bufs=3, name="q_ps_1")
        kv_ps0 = psum.tile([128, 512], f32, tag="pc", bufs=3, name="kv_ps_0")
        kv_ps1 = psum.tile([128, 512], f32, tag="pc", bufs=3, name="kv_ps_1")
        for dt_i in range(DT):
            st_flag = (dt_i == 0)
            sp_flag = (dt_i == DT - 1)
            nc.tensor.matmul(q_ps0, xs0[:, dt_i, :], wq_sb[:, dt_i, 0:512],
                             start=st_flag, stop=sp_flag)
            nc.tensor.matmul(kv_ps0[:, 0:256], xs0[:, dt_i, :],
                             wq_sb[:, dt_i, 512:768], start=st_flag, stop=sp_flag)
            nc.tensor.matmul(q_ps1, xs1[:, dt_i, :], wq_sb[:, dt_i, 0:512],
                             start=st_flag, stop=sp_flag)
            nc.tensor.matmul(kv_ps1[:, 0:256], xs1[:, dt_i, :],
                             wq_sb[:, dt_i, 512:768], start=st_flag, stop=sp_flag)

        qk_fin = {}
        qk_fin[0] = (pp_q(0, q_ps0), pp_k(0, kv_ps0))
        qk_fin[1] = (pp_q(1, q_ps1), pp_k(1, kv_ps1))

        # s-tiles 2..15 sequential; postproc compute (ACT/DVE) emitted right
        # after each tile's matmuls, PE transposes one tile behind. For the
        # last tile the q part is deferred past attention q-chunk 0 so the
        # seam isn't serialized on it.
        for st in range(2, ST):
            if st == 2:
                xs = xs2
            else:
                xs = xpool.tile([128, DT, 128], bf16, name="xs")
                nc.sync.dma_start(out=xs, in_=xt_d[st])

            q_ps = psum.tile([128, 512], f32, tag="pa", bufs=3, name=f"q_ps_{st}")
            kv_ps = psum.tile([128, 512], f32, tag="pc", bufs=3, name=f"kv_ps_{st}")
            for dt_i in range(DT):
                nc.tensor.matmul(q_ps, xs[:, dt_i, :], wq_sb[:, dt_i, 0:512],
                                 start=(dt_i == 0), stop=(dt_i == DT - 1))
                nc.tensor.matmul(kv_ps[:, 0:256], xs[:, dt_i, :],
                                 wq_sb[:, dt_i, 512:768],
                                 start=(dt_i == 0), stop=(dt_i == DT - 1))
            if st == 2:
                pp_transpose(0, *qk_fin[0])
                pp_transpose(1, *qk_fin[1])
            else:
                pp_transpose(st - 1, *qk_fin[st - 1])
                del qk_fin[st - 1]
            if st < ST - 1:
                qk_fin[st] = (pp_q(st, q_ps), pp_k(st, kv_ps))
            else:
                kfin_last = pp_k(st, kv_ps)
                q_ps_last = q_ps

        # ================= stage 3 weights (aliased over wq, 4 chunks) =====
        wo_sb = persist.tile([128, HG, D], bf16, tag="bigw")
        wo_r = wo_d.rearrange("(h p) e -> p h e", p=128)
        for ec2 in range(4):
            nc.sync.dma_start(out=wo_sb[:, :, ec2 * 1024:(ec2 + 1) * 1024],
                              in_=wo_r[:, :, ec2 * 1024:(ec2 + 1) * 1024])

        # ---- out-projection "units": generator yields after each matmul so
        # single matmuls can be interleaved into the attention loop as PE
        # filler while the scalar engine works through the exp backlog.
        PSUM_BUFS = {"pd": 2, "pa": 3, "pc": 3}

        def wout_units(st, tags=("pd",)):
            out_sb = opool.tile([128, D], bf16, name="out_sb")
            for ec in range(EC):
                tag = tags[ec % len(tags)]
                o_ps = psum.tile([128, 512], f32, tag=tag,
                                 bufs=PSUM_BUFS[tag], name="o_ps")
                for h in range(HG):
                    nc.tensor.matmul(o_ps,
                                     oT_sb[:, h, st * 128:(st + 1) * 128],
                                     wo_sb[:, h, ec * 512:(ec + 1) * 512],
                                     start=(h == 0), stop=(h == HG - 1))
                    if h < HG - 1:
                        yield
                nc.vector.tensor_copy(
                    out=out_sb[:, ec * 512:(ec + 1) * 512], in_=o_ps)
                if st == ST - 1:
                    if ec % 2 == 1:
                        nc.sync.dma_start(
                            out=out_d[st * 128:(st + 1) * 128,
                                      (ec - 1) * 512:(ec + 1) * 512],
                            in_=out_sb[:, (ec - 1) * 512:(ec + 1) * 512])
                elif ec == 3 or ec == 7:
                    half = ec // 4
                    nc.sync.dma_start(
                        out=out_d[st * 128:(st + 1) * 128,
                                  half * 2048:(half + 1) * 2048],
                        in_=out_sb[:, half * 2048:(half + 1) * 2048])
                yield

        from collections import deque
        pending = deque()

        def emit_units(n):
            k = 0
            while k < n and pending:
                try:
                    next(pending[0])
                    k += 1
                except StopIteration:
                    pending.popleft()

        # deferred per-head softmax finish: partition-reduce the accumulated
        # exp sums (ones-matmul), reciprocal, scale pv into oT.
        fin_q = deque()

        def mk_finish(h, qc, pv, acc):
            def f():
                # acc is already bf16: ones-matmul reduces it over partitions
                # directly; reciprocal and the pv scale read PSUM -> DVE.
                den_ps = psum.tile([128, 512], f32, tag="pd", bufs=2, name="den")
                nc.tensor.matmul(den_ps, onesm_sb, acc, start=True, stop=True)
                rden = scratch.tile([128, 512], f32, tag="rden")
                nc.vector.reciprocal_approx_fast(out=rden, in_=den_ps)
                nc.vector.tensor_mul(oT_sb[:, h, qc * 512:(qc + 1) * 512],
                                     pv, rden)
            return f

        # ================= stage 2+3 fused ================================
        for qc in range(QC):
            for hp in range(HG // 2):
                hh0 = 2 * hp
                pv0 = psum.tile([128, 512], f32, tag="pa", bufs=3, name=f"pv0_{qc}_{hp}")
                pv1 = psum.tile([128, 512], f32, tag="pa", bufs=3, name=f"pv1_{qc}_{hp}")
                pvs = [pv0, pv1]
                # bf16 accumulators: positive growing sums round benignly
                # (measured den err ~1e-4) and packed bf16 gets the 2x/4x
                # DVE modes, halving the per-iteration accumulate cost.
                accs = [apool.tile([128, 512], bf16, name="acc")
                        for hi in range(2)]
                n_kt = 4 * qc + 4
                for kt in range(n_kt):
                    j = kt - 4 * qc
                    off = 0 if j < 0 else 128 * j
                    exs = []
                    for hi in range(2):
                        sc_ps = psum.tile([128, 512], f32, tag="pc", bufs=3,
                                          name=f"sc_{qc}_{hp}_{kt}_{hi}")
                        nc.tensor.matmul(
                            sc_ps[:, off:512],
                            kT_sb[:, kt * 128:(kt + 1) * 128],
                            qT_sb[:, qc, hh0 + hi, off:512],
                            start=True, stop=True)
                        ex = epool.tile([128, 512], bf16, name=f"ex_{hi}")
                        nc.scalar.activation(out=ex[:, off:512],
                                             in_=sc_ps[:, off:512], func=AF.Exp)
                        if j >= 0:
                            # diag mask on the otherwise-idle GPSIMD (all
                            # SBUF operands) to keep it off the DVE chain
                            nc.gpsimd.tensor_mul(ex[:, off:off + 128],
                                                 ex[:, off:off + 128],
                                                 mask_sb)
                        if kt == 0:
                            nc.vector.tensor_copy(out=accs[hi], in_=ex)
                        else:
                            nc.vector.tensor_add(accs[hi][:, off:512],
                                                 accs[hi][:, off:512],
                                                 ex[:, off:512])
                        exs.append(ex)
                    for hi in range(2):
                        nc.tensor.matmul(pvs[hi][:, off:512], v_sb[:, kt, :],
                                         exs[hi][:, off:512],
                                         start=(kt == 0), stop=(kt == n_kt - 1))
                    # pop pending per-head finishes mid-block (kt 2/3, clear
                    # of the kt0/1 accumulator-init ops), then fill the PE
                    # with out-proj matmuls while ACT works through the exps.
                    if 2 <= kt <= 3 and fin_q:
                        fin_q.popleft()()
                    if hp == 0 and kt == 4 and qc >= 1:
                        for st in range(4 * (qc - 1), 4 * qc):
                            pending.append(wout_units(st))
                    emit_units(2 + max(0, j))
                if qc == 0 and hp == 0:
                    # deferred last stage-1 q-postproc + transpose: nothing
                    # before q-chunk 3 reads s-tile 15.
                    qfin_last = pp_q(ST - 1, q_ps_last)
                    pp_transpose(ST - 1, qfin_last, kfin_last)
                for hi in range(2):
                    fin_q.append(mk_finish(hh0 + hi, qc, pvs[hi], accs[hi]))

        while fin_q:
            fin_q.popleft()()
        # drain: attention psum tags are free now, cycle o_ps across all
        # three rings so chunk allocation never waits on a copy.
        for st in range(4 * (QC - 1), 4 * QC):
            pending.append(wout_units(st, tags=("pd", "pa", "pc")))
        emit_units(1 << 30)


    nc.compile()
    return nc


def _host_prep(x, w_qkv, w_out, q_ln_w, k_ln_w):
    """Build per-core input maps (host-side shard + transform)."""
    import ml_dtypes
    bf16 = ml_dtypes.bfloat16

    x2 = np.asarray(x, np.float32).reshape(S, D)
    # x tiles [st, d_local, d_tile, s_local] so each s-tile DMA is contiguous
    xt = np.ascontiguousarray(
        x2.reshape(ST, 128, DT, 128).transpose(0, 3, 2, 1)).astype(bf16)

    # rope tables (duplicated cos / sign-baked sin, interleaved layout)
    freqs = 1.0 / (THETA ** (np.arange(0, HD, 2, dtype=np.float64) / HD))
    ang = np.arange(S, dtype=np.float64)[:, None] * freqs[None, :]
    cos = np.cos(ang).astype(np.float32)
    sin = np.sin(ang).astype(np.float32)
    ccd = np.repeat(cos, 2, axis=1).astype(np.float16)    # [S, 128]
    ssd = np.stack([-sin, sin], axis=-1).reshape(S, HD).astype(np.float16)

    kq = np.arange(128)
    dmask = (kq[:, None] <= kq[None, :]).astype(bf16)     # [k, q]
    ident = np.eye(128, dtype=bf16)

    wq = np.asarray(w_qkv, np.float32)
    wo = np.asarray(w_out, np.float32)
    qw = np.asarray(q_ln_w, np.float32)
    kw = np.asarray(k_ln_w, np.float32)

    in_maps = []
    for g in range(N_CORES):
        wq_g = wq[512 * g:512 * (g + 1), :].reshape(HG, HD, D) * qw[None, :, None]
        wk_g = wq[D + 128 * g:D + 128 * (g + 1), :] * kw[:, None]
        wv_g = wq[D + G * HD + 128 * g:D + G * HD + 128 * (g + 1), :]
        wqkv_g = np.concatenate([wq_g.reshape(512, D), wk_g, wv_g], axis=0)
        wqkvT_g = np.ascontiguousarray(wqkv_g.T).astype(bf16)     # [D, 768]
        woT_g = np.ascontiguousarray(wo[:, 512 * g:512 * (g + 1)].T).astype(bf16)
        in_maps.append({
            "xt": xt,
            "wqkvT": wqkvT_g,
            "woT": woT_g,
            "ccd": ccd,
            "ssd": ssd,
            "dmask": dmask,
            "ident": ident,
        })
    return in_maps


_CACHE = {}


def _get_compiled():
    if "nc" not in _CACHE:
        _ensure_ntff_hook()
        _CACHE["nc"] = _build_nc()
    return _CACHE["nc"]


def run_sharded(x, w_qkv, w_out, q_ln_w, k_ln_w, trace=False):
    from concourse.bass_utils import run_bass_kernel_spmd
    nc = _get_compiled()
    in_maps = _host_prep(x, w_qkv, w_out, q_ln_w, k_ln_w)
    res = run_bass_kernel_spmd(nc, in_maps, core_ids=list(range(N_CORES)),
                               trace=trace)
    acc = np.zeros((S, D), np.float32)
    for i in range(N_CORES):
        acc += np.asarray(res.results[i]["out"], np.float32)
    return acc.reshape(1, S, D), res


def kernel(x, w_qkv, w_out, q_ln_w, k_ln_w):
    out, _ = run_sharded(x, w_qkv, w_out, q_ln_w, k_ln_w, trace=False)
    return out


# revision 46
# speedup vs baseline: 1.0107x; 1.0055x over previous
"""Trainium2 Bass kernel for GQA multi-head attention block (nn_MHA_68831145886222).

Computation (reference):
  qkv = x @ w_qkv.T ; split q[32 heads],k[8],v[8] (HD=128)
  q,k = rmsnorm(head_dim) -> rope(interleaved, theta=1e6)
  out = causal GQA attention (4 q heads per kv head)
  y   = (attn out) @ w_out.T

Sharding: tensor-parallel by kv-head group. Core g of 8 owns q heads
4g..4g+3 and kv head g (columns of the qkv projection), plus the matching
512 input rows of w_out. Each core computes a partial y [2048,4096]; the
host sums the 8 partials (bf16 partials, f32 host accumulate).

v2 changes vs v1 (423us):
  - softmax denominator no longer uses a ones-matmul per k-tile (was ~8% of
    all PE cycles): exp tiles are accumulated on the vector engine into an
    f32 per-partition partial sum; one ones-matmul per (head, q-chunk)
    reduces over partitions at the end.
  - stage-1 ramp: s-tiles 0/1 are computed interleaved per d-tile, with
    x/wq DMAs issued in consumption order, so the first matmul starts at
    ~7us instead of ~22us.
  - w_out is DMA'd in 4 e-chunks (aliased over the wq SBUF region) so the
    first out-proj tile doesn't wait for the full 4MB load.
  - output written as bf16 (halves out DMA; host sums in f32), final
    s-tile's DMA split in quarters to shrink the drain tail.
"""

import os
import sys
import types

import numpy as np

H = 32
G = 8
HD = 128
S = 2048
D = 4096
HG = H // G  # q heads per kv head = 4
EPS = 1e-5
THETA = 1e6
N_CORES = 8
ST = S // 128  # 16 s-tiles
DT = D // 128  # 32 d-tiles
QC = 4  # q chunks of 512
EC = 8  # e chunks of 512 in final matmul


def _ensure_ntff_hook():
    """Register the axon NTFF profile hook if the image's antenv lacks it,
    so run_bass_kernel_spmd(trace=True) can return exec_time_ns."""
    try:
        from antenv.axon_hooks import get_axon_ntff_profile_hook  # noqa: F401
        return
    except ImportError:
        pass
    try:
        import antenv
        mod = types.ModuleType("antenv.axon_hooks")
        _h = [None]
        mod.set_axon_ntff_profile_hook = lambda h: _h.__setitem__(0, h)
        mod.get_axon_ntff_profile_hook = lambda: _h[0]
        sys.modules["antenv.axon_hooks"] = mod
        antenv.axon_hooks = mod
        from trn_agent_boot.trn_boot import _ntff_profile_via_ctypes
        so = "/opt/axon/libaxon_pjrt.so"
        if os.path.exists(so):
            mod.set_axon_ntff_profile_hook(_ntff_profile_via_ctypes(so))
    except Exception:
        pass


def _build_nc():
    import concourse.bass as bass  # noqa: F401
    import concourse.tile as tile
    from concourse import bacc, mybir

    bf16 = mybir.dt.bfloat16
    f16 = mybir.dt.float16
    f32 = mybir.dt.float32
    AF = mybir.ActivationFunctionType

    nc = bacc.Bacc("TRN2", target_bir_lowering=False, debug=False,
                   num_devices=N_CORES)

    # ---- DRAM I/O ----
    xt_d = nc.dram_tensor("xt", [ST, 128, DT, 128], bf16, kind="ExternalInput").ap()
    wqkv_d = nc.dram_tensor("wqkvT", [D, 768], bf16, kind="ExternalInput").ap()
    wo_d = nc.dram_tensor("woT", [512, D], bf16, kind="ExternalInput").ap()
    ccd_d = nc.dram_tensor("ccd", [S, 128], f16, kind="ExternalInput").ap()
    ssd_d = nc.dram_tensor("ssd", [S, 128], f16, kind="ExternalInput").ap()
    mask_d = nc.dram_tensor("dmask", [128, 128], bf16, kind="ExternalInput").ap()
    ident_d = nc.dram_tensor("ident", [128, 128], bf16, kind="ExternalInput").ap()
    out_d = nc.dram_tensor("out", [S, D], bf16, kind="ExternalOutput").ap()

    from contextlib import ExitStack
    with tile.TileContext(nc) as tc, ExitStack() as ctx:
        const = ctx.enter_context(tc.tile_pool(name="const", bufs=1))
        persist = ctx.enter_context(tc.tile_pool(name="persist", bufs=1))
        xpool = ctx.enter_context(tc.tile_pool(name="xpool", bufs=3))
        scratch = ctx.enter_context(tc.tile_pool(name="scratch", bufs=2))
        small = ctx.enter_context(tc.tile_pool(name="small", bufs=2))
        epool = ctx.enter_context(tc.tile_pool(name="epool", bufs=9))
        apool = ctx.enter_context(tc.tile_pool(name="apool", bufs=4))
        opool = ctx.enter_context(tc.tile_pool(name="opool", bufs=2))
        psum = ctx.enter_context(tc.tile_pool(name="psum", bufs=4, space="PSUM"))

        # ---- DMA issue order = consumption order: s-tiles 0/1 compute
        # interleaved per d-tile group so the PE starts as soon as the first
        # x chunk + wq d-slice land.
        wq_sb = persist.tile([128, DT, 768], bf16, tag="bigw")
        wq_r = wqkv_d.rearrange("(t p) e -> p t e", p=128)

        xs0 = xpool.tile([128, DT, 128], bf16, name="xs")
        xs1 = xpool.tile([128, DT, 128], bf16, name="xs")
        xs2 = xpool.tile([128, DT, 128], bf16, name="xs")
        nc.sync.dma_start(out=xs0[:, 0:4, :], in_=xt_d[0, :, 0:4, :])
        nc.sync.dma_start(out=wq_sb[:, 0:1, :], in_=wq_r[:, 0:1, :])
        nc.sync.dma_start(out=xs1[:, 0:4, :], in_=xt_d[1, :, 0:4, :])
        nc.sync.dma_start(out=wq_sb[:, 1:2, :], in_=wq_r[:, 1:2, :])
        nc.sync.dma_start(out=wq_sb[:, 2:4, :], in_=wq_r[:, 2:4, :])

        ccd_sb = const.tile([128, ST, 128], f16)
        ssd_sb = const.tile([128, ST, 128], f16)
        mask_sb = const.tile([128, 128], bf16)
        ident_sb = const.tile([128, 128], bf16)
        for g in range(4, DT, 4):
            nc.sync.dma_start(out=xs0[:, g:g + 4, :], in_=xt_d[0, :, g:g + 4, :])
            nc.sync.dma_start(out=xs1[:, g:g + 4, :], in_=xt_d[1, :, g:g + 4, :])
            nc.sync.dma_start(out=wq_sb[:, g:g + 4, :], in_=wq_r[:, g:g + 4, :])
        nc.sync.dma_start(out=xs2[:, 0:16, :], in_=xt_d[2, :, 0:16, :])
        nc.sync.dma_start(out=xs2[:, 16:32, :], in_=xt_d[2, :, 16:32, :])
        nc.sync.dma_start(out=ccd_sb,
                          in_=ccd_d.rearrange("(t p) h -> p t h", p=128))
        nc.sync.dma_start(out=ssd_sb,
                          in_=ssd_d.rearrange("(t p) h -> p t h", p=128))
        nc.sync.dma_start(out=mask_sb, in_=mask_d)
        nc.sync.dma_start(out=ident_sb, in_=ident_d)

        onesm_sb = const.tile([128, 128], bf16)
        nc.vector.memset(onesm_sb, 1.0)
        bias_q = const.tile([128, 1], f32)
        nc.vector.memset(bias_q, float(HD * EPS))
        bias_k = const.tile([128, 1], f32)
        nc.vector.memset(bias_k, float(EPS))

        qT_sb = persist.tile([128, QC, HG, 512], bf16)  # [hd, qchunk, head, s%512]
        kT_sb = persist.tile([128, S], bf16)       # [hd, s]
        v_sb = persist.tile([128, ST, 128], bf16)  # [s_local, s_tile, hd]
        oT_sb = persist.tile([128, HG, S], bf16)   # attn outT [hd, head, s]

        def pp_k(st, kv_ps):
            # k/v part of the postproc: v copy, k rmsnorm + rope -> kfin.
            # split from the q part so the last s-tile can release its kv
            # psum slot before attention q-chunk 0 is emitted.
            nc.vector.tensor_copy(out=v_sb[:, st, :], in_=kv_ps[:, 128:256])
            ssq_k = small.tile([128, 1], f32)
            sqk = small.tile([128, 128], f32)
            nc.scalar.activation(out=sqk, in_=kv_ps[:, 0:128], func=AF.Square,
                                 accum_out=ssq_k)
            rstd_k = small.tile([128, 1], f32)
            nc.scalar.activation(out=rstd_k, in_=ssq_k,
                                 func=AF.Sqrt, bias=bias_k, scale=1.0 / HD)
            nc.vector.reciprocal(out=rstd_k, in_=rstd_k)

            k1 = kv_ps[:, 0:128].rearrange("p (r two) -> p r two", two=2)
            rot_k = small.tile([128, 64, 2], f32)
            nc.vector.tensor_copy(out=rot_k, in_=k1[:, :, ::-1])
            kcc = small.tile([128, 128], f32)
            nc.vector.tensor_mul(kcc, kv_ps[:, 0:128], ccd_sb[:, st, :])
            kss = small.tile([128, 128], f32)
            nc.vector.tensor_mul(kss, rot_k.rearrange("p r two -> p (r two)"),
                                 ssd_sb[:, st, :])
            krope = small.tile([128, 128], f32)
            nc.vector.tensor_add(krope, kcc, kss)
            kfin = small.tile([128, 128], bf16)
            nc.vector.tensor_scalar_mul(kfin, krope, rstd_k)
            return kfin

        def pp_q(st, q_ps):
            # q part: per-head rmsnorm (with folded score scale) + rope
            ssq = small.tile([128, 4], f32)
            sqs = scratch.tile([128, 512], f32)
            for hh in range(HG):
                nc.scalar.activation(out=sqs[:, hh * 128:(hh + 1) * 128],
                                     in_=q_ps[:, hh * 128:(hh + 1) * 128],
                                     func=AF.Square,
                                     accum_out=ssq[:, hh:hh + 1])
            rstd = small.tile([128, 4], f32)
            nc.scalar.activation(out=rstd, in_=ssq,
                                 func=AF.Sqrt, bias=bias_q, scale=1.0)
            nc.vector.reciprocal(out=rstd, in_=rstd)

            q4 = q_ps.rearrange("p (h r two) -> p h r two", h=HG, two=2)
            rot_q = scratch.tile([128, HG, 64, 2], f32)
            nc.vector.tensor_copy(out=rot_q, in_=q4[:, :, :, ::-1])
            cc_b = ccd_sb[:, st, :].unsqueeze(1).broadcast_to((128, HG, 128))
            ss_b = ssd_sb[:, st, :].unsqueeze(1).broadcast_to((128, HG, 128))
            qcc = scratch.tile([128, HG, 128], f32)
            nc.vector.tensor_mul(qcc, q_ps.rearrange("p (h e) -> p h e", h=HG), cc_b)
            qss = scratch.tile([128, HG, 128], f32)
            nc.vector.tensor_mul(qss, rot_q.rearrange("p h r two -> p h (r two)"), ss_b)
            qrope = scratch.tile([128, HG, 128], f32)
            nc.vector.tensor_add(qrope, qcc, qss)
            qfin = scratch.tile([128, HG, 128], bf16)
            for hh in range(HG):
                nc.vector.tensor_scalar_mul(qfin[:, hh, :], qrope[:, hh, :],
                                            rstd[:, hh:hh + 1])
            return qfin

        def pp_transpose(st, qfin, kfin):
            # transpose q heads and k into [hd, s] layout (PE); emitted one
            # s-tile behind the matmul blocks so the PE never waits on qfin.
            # qT layout groups the 4 heads of each q-chunk adjacently so a
            # score matmul pair for two heads can share one psum tile.
            qc_i, so = st // 4, (st % 4) * 128
            for hh in range(HG):
                tq_ps = psum.tile([128, 128], bf16, tag="pd", bufs=2)
                nc.tensor.transpose(tq_ps, qfin[:, hh, :], ident_sb)
                nc.scalar.copy(out=qT_sb[:, qc_i, hh, so:so + 128], in_=tq_ps)
            tk_ps = psum.tile([128, 128], bf16, tag="pd", bufs=2)
            nc.tensor.transpose(tk_ps, kfin, ident_sb)
            nc.scalar.copy(out=kT_sb[:, st * 128:(st + 1) * 128], in_=tk_ps)

        # ================= stage 1: qkv projection + postproc ==============
        # s-tiles 0/1 interleaved per d-tile (DMA-paced ramp)
        q_ps0 = psum.tile([128, 512], f32, tag="pa", bufs=3, name="q_ps_0")
        q_ps1 = psum.tile([128, 512], f32, tag="pa", bufs=3, name="q_ps_1")
        kv_ps0 = psum.tile([128, 512], f32, tag="pc", bufs=3, name="kv_ps_0")
        kv_ps1 = psum.tile([128, 512], f32, tag="pc", bufs=3, name="kv_ps_1")
        for dt_i in range(DT):
            st_flag = (dt_i == 0)
            sp_flag = (dt_i == DT - 1)
            nc.tensor.matmul(q_ps0, xs0[:, dt_i, :], wq_sb[:, dt_i, 0:512],
                             start=st_flag, stop=sp_flag)
            nc.tensor.matmul(kv_ps0[:, 0:256], xs0[:, dt_i, :],
                             wq_sb[:, dt_i, 512:768], start=st_flag, stop=sp_flag)
            nc.tensor.matmul(q_ps1, xs1[:, dt_i, :], wq_sb[:, dt_i, 0:512],
                             start=st_flag, stop=sp_flag)
            nc.tensor.matmul(kv_ps1[:, 0:256], xs1[:, dt_i, :],
                             wq_sb[:, dt_i, 512:768], start=st_flag, stop=sp_flag)

        qk_fin = {}
        qk_fin[0] = (pp_q(0, q_ps0), pp_k(0, kv_ps0))
        qk_fin[1] = (pp_q(1, q_ps1), pp_k(1, kv_ps1))

        # s-tiles 2..15 sequential; postproc compute (ACT/DVE) emitted right
        # after each tile's matmuls, PE transposes one tile behind. For the
        # last tile the q part is deferred past attention q-chunk 0 so the
        # seam isn't serialized on it.
        for st in range(2, ST):
            if st == 2:
                xs = xs2
            else:
                xs = xpool.tile([128, DT, 128], bf16, name="xs")
                nc.sync.dma_start(out=xs, in_=xt_d[st])

            q_ps = psum.tile([128, 512], f32, tag="pa", bufs=3, name=f"q_ps_{st}")
            kv_ps = psum.tile([128, 512], f32, tag="pc", bufs=3, name=f"kv_ps_{st}")
            for dt_i in range(DT):
                nc.tensor.matmul(q_ps, xs[:, dt_i, :], wq_sb[:, dt_i, 0:512],
                                 start=(dt_i == 0), stop=(dt_i == DT - 1))
                nc.tensor.matmul(kv_ps[:, 0:256], xs[:, dt_i, :],
                                 wq_sb[:, dt_i, 512:768],
                                 start=(dt_i == 0), stop=(dt_i == DT - 1))
            if st == 2:
                pp_transpose(0, *qk_fin[0])
                pp_transpose(1, *qk_fin[1])
            else:
                pp_transpose(st - 1, *qk_fin[st - 1])
                del qk_fin[st - 1]
            if st < ST - 1:
                qk_fin[st] = (pp_q(st, q_ps), pp_k(st, kv_ps))
            else:
                kfin_last = pp_k(st, kv_ps)
                q_ps_last = q_ps

        # ================= stage 3 weights (aliased over wq, 4 chunks) =====
        wo_sb = persist.tile([128, HG, D], bf16, tag="bigw")
        wo_r = wo_d.rearrange("(h p) e -> p h e", p=128)
        for ec2 in range(4):
            nc.sync.dma_start(out=wo_sb[:, :, ec2 * 1024:(ec2 + 1) * 1024],
                              in_=wo_r[:, :, ec2 * 1024:(ec2 + 1) * 1024])

        # ---- out-projection "units": generator yields after each matmul so
        # single matmuls can be interleaved into the attention loop as PE
        # filler while the scalar engine works through the exp backlog.
        PSUM_BUFS = {"pd": 2, "pa": 3, "pc": 3}

        def wout_units(st, tags=("pd",)):
            out_sb = opool.tile([128, D], bf16, name="out_sb")
            for ec in range(EC):
                tag = tags[ec % len(tags)]
                o_ps = psum.tile([128, 512], f32, tag=tag,
                                 bufs=PSUM_BUFS[tag], name="o_ps")
                for h in range(HG):
                    nc.tensor.matmul(o_ps,
                                     oT_sb[:, h, st * 128:(st + 1) * 128],
                                     wo_sb[:, h, ec * 512:(ec + 1) * 512],
                                     start=(h == 0), stop=(h == HG - 1))
                    if h < HG - 1:
                        yield
                nc.vector.tensor_copy(
                    out=out_sb[:, ec * 512:(ec + 1) * 512], in_=o_ps)
                if st == ST - 1:
                    if ec % 2 == 1:
                        nc.sync.dma_start(
                            out=out_d[st * 128:(st + 1) * 128,
                                      (ec - 1) * 512:(ec + 1) * 512],
                            in_=out_sb[:, (ec - 1) * 512:(ec + 1) * 512])
                elif ec == 3 or ec == 7:
                    half = ec // 4
                    nc.sync.dma_start(
                        out=out_d[st * 128:(st + 1) * 128,
                                  half * 2048:(half + 1) * 2048],
                        in_=out_sb[:, half * 2048:(half + 1) * 2048])
                yield

        from collections import deque
        pending = deque()

        def emit_units(n):
            k = 0
            while k < n and pending:
                try:
                    next(pending[0])
                    k += 1
                except StopIteration:
                    pending.popleft()

        # deferred per-head softmax finish: partition-reduce the accumulated
        # exp sums (ones-matmul), reciprocal, scale pv into oT.
        fin_q = deque()

        def mk_finish(h, qc, pv, acc):
            def f():
                # acc is already bf16: ones-matmul reduces it over partitions
                # directly; reciprocal and the pv scale read PSUM -> DVE.
                den_ps = psum.tile([128, 512], f32, tag="pd", bufs=2, name="den")
                nc.tensor.matmul(den_ps, onesm_sb, acc, start=True, stop=True)
                rden = scratch.tile([128, 512], f32, tag="rden")
                nc.vector.reciprocal_approx_fast(out=rden, in_=den_ps)
                nc.vector.tensor_mul(oT_sb[:, h, qc * 512:(qc + 1) * 512],
                                     pv, rden)
            return f

        # ================= stage 2+3 fused ================================
        for qc in range(QC):
            for hp in range(HG // 2):
                hh0 = 2 * hp
                pv0 = psum.tile([128, 512], f32, tag="pa", bufs=3, name=f"pv0_{qc}_{hp}")
                pv1 = psum.tile([128, 512], f32, tag="pa", bufs=3, name=f"pv1_{qc}_{hp}")
                pvs = [pv0, pv1]
                # bf16 accumulators: positive growing sums round benignly
                # (measured den err ~1e-4) and packed bf16 gets the 2x/4x
                # DVE modes, halving the per-iteration accumulate cost.
                accs = [apool.tile([128, 512], bf16, name="acc")
                        for hi in range(2)]
                n_kt = 4 * qc + 4
                for kt in range(n_kt):
                    j = kt - 4 * qc
                    off = 0 if j < 0 else 128 * j
                    exs = []
                    for hi in range(2):
                        sc_ps = psum.tile([128, 512], f32, tag="pc", bufs=3,
                                          name=f"sc_{qc}_{hp}_{kt}_{hi}")
                        nc.tensor.matmul(
                            sc_ps[:, off:512],
                            kT_sb[:, kt * 128:(kt + 1) * 128],
                            qT_sb[:, qc, hh0 + hi, off:512],
                            start=True, stop=True)
                        ex = epool.tile([128, 512], bf16, name=f"ex_{hi}")
                        nc.scalar.activation(out=ex[:, off:512],
                                             in_=sc_ps[:, off:512], func=AF.Exp)
                        if j >= 0:
                            # diag mask on the otherwise-idle GPSIMD (all
                            # SBUF operands) to keep it off the DVE chain
                            nc.gpsimd.tensor_mul(ex[:, off:off + 128],
                                                 ex[:, off:off + 128],
                                                 mask_sb)
                        if kt == 0:
                            nc.vector.tensor_copy(out=accs[hi], in_=ex)
                        else:
                            nc.vector.tensor_add(accs[hi][:, off:512],
                                                 accs[hi][:, off:512],
                                                 ex[:, off:512])
                        exs.append(ex)
                    for hi in range(2):
                        nc.tensor.matmul(pvs[hi][:, off:512], v_sb[:, kt, :],
                                         exs[hi][:, off:512],
                                         start=(kt == 0), stop=(kt == n_kt - 1))
                    # pop pending per-head finishes mid-block (kt 2/3, clear
                    # of the kt0/1 accumulator-init ops), then fill the PE
                    # with out-proj matmuls while ACT works through the exps.
                    if 2 <= kt <= 3 and fin_q:
                        fin_q.popleft()()
                    if hp == 0 and kt == 4 and qc >= 1:
                        for st in range(4 * (qc - 1), 4 * qc):
                            pending.append(wout_units(st))
                    emit_units(2 + max(0, j))
                if qc == 0 and hp == 0:
                    # deferred last stage-1 q-postproc + transpose: nothing
                    # before q-chunk 3 reads s-tile 15.
                    qfin_last = pp_q(ST - 1, q_ps_last)
                    pp_transpose(ST - 1, qfin_last, kfin_last)
                for hi in range(2):
                    fin_q.append(mk_finish(hh0 + hi, qc, pvs[hi], accs[hi]))

        while fin_q:
            fin_q.popleft()()
        # drain: attention psum tags are free now, cycle o_ps across all
        # three rings so chunk allocation never waits on a copy.
        for st in range(4 * (QC - 1), 4 * QC):
            pending.append(wout_units(st, tags=("pd", "pa", "pc")))
        emit_units(1 << 30)


    nc.compile()
    return nc


def _host_prep(x, w_qkv, w_out, q_ln_w, k_ln_w):
    """Build per-core input maps (host-side shard + transform)."""
    import ml_dtypes
    bf16 = ml_dtypes.bfloat16

    x2 = np.asarray(x, np.float32).reshape(S, D)
    # x tiles [st, d_local, d_tile, s_local] so each s-tile DMA is contiguous
    xt = np.ascontiguousarray(
        x2.reshape(ST, 128, DT, 128).transpose(0, 3, 2, 1)).astype(bf16)

    # rope tables (duplicated cos / sign-baked sin, interleaved layout)
    freqs = 1.0 / (THETA ** (np.arange(0, HD, 2, dtype=np.float64) / HD))
    ang = np.arange(S, dtype=np.float64)[:, None] * freqs[None, :]
    cos = np.cos(ang).astype(np.float32)
    sin = np.sin(ang).astype(np.float32)
    ccd = np.repeat(cos, 2, axis=1).astype(np.float16)    # [S, 128]
    ssd = np.stack([-sin, sin], axis=-1).reshape(S, HD).astype(np.float16)

    kq = np.arange(128)
    dmask = (kq[:, None] <= kq[None, :]).astype(bf16)     # [k, q]
    ident = np.eye(128, dtype=bf16)

    wq = np.asarray(w_qkv, np.float32)
    wo = np.asarray(w_out, np.float32)
    qw = np.asarray(q_ln_w, np.float32)
    kw = np.asarray(k_ln_w, np.float32)

    in_maps = []
    for g in range(N_CORES):
        wq_g = wq[512 * g:512 * (g + 1), :].reshape(HG, HD, D) * qw[None, :, None]
        wk_g = wq[D + 128 * g:D + 128 * (g + 1), :] * kw[:, None]
        wv_g = wq[D + G * HD + 128 * g:D + G * HD + 128 * (g + 1), :]
        wqkv_g = np.concatenate([wq_g.reshape(512, D), wk_g, wv_g], axis=0)
        wqkvT_g = np.ascontiguousarray(wqkv_g.T).astype(bf16)     # [D, 768]
        woT_g = np.ascontiguousarray(wo[:, 512 * g:512 * (g + 1)].T).astype(bf16)
        in_maps.append({
            "xt": xt,
            "wqkvT": wqkvT_g,
            "woT": woT_g,
            "ccd": ccd,
            "ssd": ssd,
            "dmask": dmask,
            "ident": ident,
        })
    return in_maps


_CACHE = {}


def _get_compiled():
    if "nc" not in _CACHE:
        _ensure_ntff_hook()
        _CACHE["nc"] = _build_nc()
    return _CACHE["nc"]


def run_sharded(x, w_qkv, w_out, q_ln_w, k_ln_w, trace=False):
    from concourse.bass_utils import run_bass_kernel_spmd
    nc = _get_compiled()
    in_maps = _host_prep(x, w_qkv, w_out, q_ln_w, k_ln_w)
    res = run_bass_kernel_spmd(nc, in_maps, core_ids=list(range(N_CORES)),
                               trace=trace)
    acc = np.zeros((S, D), np.float32)
    for i in range(N_CORES):
        acc += np.asarray(res.results[i]["out"], np.float32)
    return acc.reshape(1, S, D), res


def kernel(x, w_qkv, w_out, q_ln_w, k_ln_w):
    out, _ = run_sharded(x, w_qkv, w_out, q_ln_w, k_ln_w, trace=False)
    return out
